# revision 29
# baseline (speedup 1.0000x reference)
"""CharRNNEmbedding Trainium2 kernel: 2-layer biLSTM char encoder over 8 NeuronCores.

Data-parallel: 4096 words split 512/core; weights replicated. v2 design:
- All scan matmuls are fp8e4 DoubleRow (K packed 2/partition, 0.5 cyc/row);
  the ih matmul zero-pads its pair slot (stride-0 ifmap trick) so K=65 fits
  one DR instruction per M-tile.
- Gate activations write bf16 to SBUF; cell state c is bf16; h is fp8 (it
  only feeds DR matmuls). Elementwise state math runs on DVE at the 2x
  2-byte rate; the final h-mul pays 1x for its fp8 output.
- Embedding: chunked (2-4 timesteps) fp16 broadcast of ids, one is_equal
  per vocab k-tile (bf16 out, 4x DVE rate), bf16 one-hot matmul, fp8
  copies into x2. x2 column blocks live in embed order (0,15,1,14,...)
  so chunk copies stay contiguous; chunks interleave with the scan.
- Layer 1 collapses to two single LSTM cells (reference consumes only
  h1[0,:,:H] and h1[-1,:,H:], both first-step-from-zero outputs), f-gates
  and w_hh_l1* unused. Output projection kept bf16 for accuracy.
Biases fold into matmuls via constant-1 rows (all exactly representable).
"""
import sys

sys.path.insert(0, "/opt/trn_rl_repo")

import numpy as np
import ml_dtypes
from contextlib import ExitStack

import concourse.bass as bass
import concourse.tile as tile
import concourse.mybir as mybir
from concourse.bass_utils import run_bass_kernel_spmd

F32 = mybir.dt.float32
BF16 = mybir.dt.bfloat16
FP16 = mybir.dt.float16
FP8 = mybir.dt.float8e4
AF = mybir.ActivationFunctionType
ALU = mybir.AluOpType
PM = mybir.MatmulPerfMode
F8NP = ml_dtypes.float8_e4m3fn
BFNP = ml_dtypes.bfloat16

NCORES = 8
B, S, T = 32, 128, 16
VOCAB, E, H = 262, 64, 256
NC_W = B * S // NCORES          # words per core = 512
TOK = NC_W * T                  # tokens per core = 8192
G4 = 4 * H                      # 1024

# embed order: x2 column block j holds timestep EO[j]
EO = [0, 15, 1, 14, 2, 13, 3, 12, 4, 11, 5, 10, 6, 9, 7, 8]
XPOS = {t: j for j, t in enumerate(EO)}


def _pack_host(inp):
    """Pack all weights into per-dtype arrays (host-side, weights only)."""
    wih0 = np.zeros((66, 2, 2 * G4), F8NP)
    whh0 = np.zeros((128, 2, 2 * G4), F8NP)
    wih1 = np.zeros((128, 2, 2 * 2 * 768), F8NP)
    wih1_ones = np.zeros((1, 2, 2 * 768), F8NP)
    sel = np.r_[0:256, 768:1024, 512:768]       # l1 gate rows kept: i, o, g
    for d, nm in enumerate("fb"):
        w = np.asarray(inp[f"w_ih_l0{nm}"], np.float32)       # [1024, 64]
        b = np.asarray(inp[f"b_l0{nm}"], np.float32)          # [1024]
        aug = np.concatenate([w.T, b[None, :]], 0)            # [65, 1024]
        wih0[:65, 0, d * G4:(d + 1) * G4] = aug.astype(F8NP)
        whh = np.asarray(inp[f"w_hh_l0{nm}"], np.float32)     # [1024, 256]
        for i in range(2):
            whh0[:, i, d * G4:(d + 1) * G4] = whh[:, i * 128:(i + 1) * 128].T.astype(F8NP)
        w1 = np.asarray(inp[f"w_ih_l1{nm}"], np.float32)[sel]  # [768, 512]
        b1 = np.asarray(inp[f"b_l1{nm}"], np.float32)[sel]     # [768]
        for kt in range(2):
            for i in range(2):
                rows = slice(kt * 256 + i * 128, kt * 256 + (i + 1) * 128)
                wih1[:, i, (d * 2 + kt) * 768:(d * 2 + kt + 1) * 768] = \
                    w1[:, rows].T.astype(F8NP)
        wih1_ones[0, 0, d * 768:(d + 1) * 768] = b1.astype(F8NP)

    wo = np.asarray(inp["w_out"], np.float32)                 # [256, 512]
    bo = np.asarray(inp["b_out"], np.float32)                 # [256]
    wout = np.zeros((128, 4 * 256), BFNP)
    for k in range(4):
        wout[:, k * 256:(k + 1) * 256] = wo[:, k * 128:(k + 1) * 128].T.astype(BFNP)
    wout_ones = bo[None, :].astype(BFNP)                      # [1, 256]

    ce = np.asarray(inp["char_emb"], np.float32)              # [262, 64]
    ce_aug = np.zeros((384, 66), np.float32)
    ce_aug[:VOCAB, :E] = ce
    ce_aug[:VOCAB, E] = 1.0
    cemb = np.zeros((128, 3 * 66), BFNP)
    for k in range(3):
        cemb[:, k * 66:(k + 1) * 66] = ce_aug[k * 128:(k + 1) * 128].astype(BFNP)
    cols = (np.arange(128)[:, None] + 128 * np.arange(3)[None, :]).astype(np.float32)
    return dict(wih0=wih0, whh0=whh0, wih1=wih1, wih1_ones=wih1_ones,
                wout=wout, wout_ones=wout_ones, cemb=cemb, cols=cols)


def _legalize_waits(nc, max_waits=1):
    """This walrus build rejects >1 sync wait per instruction: split extras
    onto standalone no-ops ahead of the instruction (same engine queue)."""
    ctr = 0
    for f in nc.m.functions:
        for blk in f.blocks:
            out = []
            for inst in blk.instructions:
                si = inst.sync_info
                if si is not None and si.on_wait and len(si.on_wait) > max_waits:
                    waits = list(si.on_wait)
                    for w in waits[:-max_waits]:
                        nop = mybir.InstNoOp(name=f"I-wsplit-{ctr}")
                        ctr += 1
                        nop.engine = inst.engine
                        nop.sync_info = mybir.SyncInfo(on_wait=[w], on_update=[])
                        out.append(nop)
                    inst.sync_info = mybir.SyncInfo(
                        on_wait=waits[-max_waits:], on_update=list(si.on_update))
                out.append(inst)
            blk.instructions = out
    return nc


def build_nc(debug=False):
    nc = bass.Bass()
    wih0_d = nc.dram_tensor("wih0", [66, 2, 2 * G4], FP8, kind="ExternalInput")
    whh0_d = nc.dram_tensor("whh0", [128, 2, 2 * G4], FP8, kind="ExternalInput")
    wih1_d = nc.dram_tensor("wih1", [128, 2, 2 * 2 * 768], FP8, kind="ExternalInput")
    wih1o_d = nc.dram_tensor("wih1_ones", [1, 2, 2 * 768], FP8, kind="ExternalInput")
    wout_d = nc.dram_tensor("wout", [128, 4 * 256], BF16, kind="ExternalInput")
    wouto_d = nc.dram_tensor("wout_ones", [1, 256], BF16, kind="ExternalInput")
    cemb_d = nc.dram_tensor("cemb", [128, 3 * 66], BF16, kind="ExternalInput")
    cols_d = nc.dram_tensor("cols", [128, 3], F32, kind="ExternalInput")
    ids_d = nc.dram_tensor("ids", [TOK], FP16, kind="ExternalInput")  # embed order
    out_d = nc.dram_tensor("out", [128, 2, NC_W], F32, kind="ExternalOutput")

    with tile.TileContext(nc) as tc, ExitStack() as ctx:
        wpool = ctx.enter_context(tc.tile_pool(name="weights", bufs=1))
        spool = ctx.enter_context(tc.tile_pool(name="state", bufs=1))
        gpool = ctx.enter_context(tc.tile_pool(name="gates", bufs=2))
        epool = ctx.enter_context(tc.tile_pool(name="embed", bufs=2))
        psum = ctx.enter_context(tc.tile_pool(name="ps", bufs=2, space="PSUM"))

        w_cols = wpool.tile([128, 3], F32)
        nc.sync.dma_start(w_cols[:], cols_d[:])
        w_ce = wpool.tile([128, 3 * 66], BF16)
        nc.sync.dma_start(w_ce[:], cemb_d[:])
        w_ih0 = wpool.tile([66, 2, 2 * G4], FP8)
        nc.sync.dma_start(w_ih0[:], wih0_d[:])
        w_hh0 = wpool.tile([128, 2, 2 * G4], FP8)
        nc.sync.dma_start(w_hh0[:], whh0_d[:])
        w_ih1 = wpool.tile([128, 2, 2 * 2 * 768], FP8)
        nc.sync.dma_start(w_ih1[:], wih1_d[:])
        w_ih1o = wpool.tile([1, 2, 2 * 768], FP8)
        nc.sync.dma_start(w_ih1o[:], wih1o_d[:])
        w_out = wpool.tile([128, 4 * 256], BF16)
        nc.sync.dma_start(w_out[:], wout_d[:])
        w_outo = wpool.tile([1, 256], BF16)
        nc.sync.dma_start(w_outo[:], wouto_d[:])

        def wih0_s(d, m):          # [66, 2, 128] DR weights, pair slot 1 = 0
            o = d * G4 + m * 128
            return w_ih0[:, :, o:o + 128]

        def whh0_s(d, m):          # [128, 2, 128]
            o = d * G4 + m * 128
            return w_hh0[:, :, o:o + 128]

        def wih1_s(d, kt, m):      # [128, 2, 128]
            o = (d * 2 + kt) * 768 + m * 128
            return w_ih1[:, :, o:o + 128]

        def wih1o_s(d, m):         # [1, 2, 128]
            o = d * 768 + m * 128
            return w_ih1o[:, :, o:o + 128]

        def wout_s(k, m):          # [128, 128]
            o = k * 256 + m * 128
            return w_out[:, o:o + 128]

        def cemb_s(k):             # [128, 66]
            return w_ce[:, k * 66:(k + 1) * 66]

        # ---- persistent state ----
        x2 = spool.tile([66, TOK], FP8)            # embed-order column blocks
        h = {d: spool.tile([128, 2, NC_W], FP8, name=f"h{d}") for d in range(2)}
        c = {d: spool.tile([128, 2, NC_W], BF16, name=f"c{d}") for d in range(2)}
        h0_f0 = spool.tile([128, 2, NC_W], FP8)    # h_fwd after t=0
        hb_15 = spool.tile([128, 2, NC_W], FP8)    # h_bwd after its first step
        merged = spool.tile([128, 4, NC_W], BF16)
        ones8 = spool.tile([1, 2, NC_W], FP8)      # DR pair (1, 0) for l1 bias
        ones_b = spool.tile([1, NC_W], BF16)       # bf16 ones row for out proj

        def x_pairs(t):
            """x2 block for timestep t as a DR ifmap [66, 2, NC_W] whose pair
            dim has stride 0 (second slot multiplies zero weights)."""
            ap = x2[:]
            return bass.AP(tensor=ap.tensor, offset=ap.offset + XPOS[t] * NC_W,
                           ap=[[ap.ap[0][0], 66], [0, 2], [1, NC_W]])

        # ---- embedding: batches of 4 timesteps through one 4-bank PSUM.
        # j-major matmul order + split copies so the first x2 blocks land
        # as early as possible (the scan starts on them). ----
        def embed_chunk(b0, nb, jorder=None, split_copy=False):
            idsB = epool.tile([128, 4 * NC_W], FP16, tag="idsB")
            src = bass.AP(tensor=ids_d[:].tensor, offset=b0 * NC_W,
                          ap=[[0, 128], [1, nb * NC_W]])
            nc.gpsimd.dma_start(idsB[:, 0:nb * NC_W], src)
            ps_e = psum.tile([128, 4, NC_W], F32, tag="gp", name=f"emb{b0}")
            ohs = []
            for k in range(3):
                oh = epool.tile([128, 4, NC_W], BF16, tag="oh", bufs=3)
                nc.vector.tensor_scalar(oh[:, 0:nb, :], idsB[:, 0:nb * NC_W],
                                        w_cols[:, k:k + 1], None, op0=ALU.is_equal)
                ohs.append(oh)
            for j in (jorder or range(nb)):
                for k in range(3):
                    nc.tensor.matmul(ps_e[0:66, j, :], cemb_s(k), ohs[k][:, j, :],
                                     start=(k == 0), stop=(k == 2))
                if split_copy:
                    o = (b0 + j) * NC_W
                    nc.vector.tensor_copy(x2[:, o:o + NC_W], ps_e[0:66, j, :])
                elif j % 2 == 1:
                    o = (b0 + j - 1) * NC_W
                    nc.vector.tensor_copy(x2[:, o:o + 2 * NC_W],
                                          ps_e[0:66, j - 1:j + 1, :])

        # ---- one scan (t, dir) unit ----
        # t=0 skips f (c starts at 0): wave A packs [i0 i1 o0 o1] under one
        # sigmoid and wave B is just [g0 g1]. At t=1 the hh matmuls read the
        # t=0 snapshot tiles directly (flush_tail wrote h there).
        def scan_dir(t, d):
            xw = x_pairs(t if d == 0 else T - 1 - t)
            h_src = (h[d] if t != 1 else (h0_f0 if d == 0 else hb_15))[:]
            mA = (0, 1, 2, 3)
            gpA = psum.tile([128, 4, NC_W], F32, tag="gp", name=f"gA{t}_{d}")
            for pos, m in enumerate(mA):
                nc.tensor.matmul(gpA[:, pos, :], wih0_s(d, m), xw,
                                 start=True, stop=(t == 0),
                                 perf_mode=PM.DoubleRow)
                if t > 0:
                    nc.tensor.matmul(gpA[:, pos, :], whh0_s(d, m), h_src,
                                     start=False, stop=True,
                                     perf_mode=PM.DoubleRow)
            na = 2 if t == 0 else 4
            sigA = gpool.tile([128, 4, NC_W], BF16, tag="sigA")
            nc.scalar.activation(sigA[:, 0:na, :], gpA[:, 0:na, :], AF.Sigmoid)
            # wave B: [g0 g1 o0 o1]
            mB = (4, 5, 6, 7)
            gpB = psum.tile([128, 4, NC_W], F32, tag="gp", name=f"gB{t}_{d}")
            for pos, m in enumerate(mB):
                nc.tensor.matmul(gpB[:, pos, :], wih0_s(d, m), xw,
                                 start=True, stop=(t == 0),
                                 perf_mode=PM.DoubleRow)
                if t > 0:
                    nc.tensor.matmul(gpB[:, pos, :], whh0_s(d, m), h_src,
                                     start=False, stop=True,
                                     perf_mode=PM.DoubleRow)
            tg = gpool.tile([128, 2, NC_W], BF16, tag="tg")
            nc.scalar.activation(tg[:], gpB[:, 0:2, :], AF.Tanh)
            sigo = gpool.tile([128, 2, NC_W], BF16, tag="sigo", bufs=3)
            nc.scalar.activation(sigo[:], gpB[:, 2:4, :], AF.Sigmoid)
            # state update (DVE, bf16)
            if t > 0:
                nc.vector.tensor_mul(c[d][:], c[d][:], sigA[:, 2:4, :])
                t1 = gpool.tile([128, 2, NC_W], BF16, tag="t1")
                nc.vector.tensor_mul(t1[:], sigA[:, 0:2, :], tg[:])
                nc.vector.tensor_add(c[d][:], c[d][:], t1[:])
            else:
                nc.vector.tensor_mul(c[d][:], sigA[:, 0:2, :], tg[:])
            return sigo

        pending = []   # deferred (tanh_c, h-mul) tails

        def flush_tail():
            if not pending:
                return
            t, d, sigo = pending.pop(0)
            tc_ = gpool.tile([128, 2, NC_W], BF16, tag="tc",
                             name=f"tc{t}_{d}")
            nc.scalar.activation(tc_[:], c[d][:], AF.Tanh)
            if t == 0:
                dst = h0_f0 if d == 0 else hb_15
                nc.vector.tensor_mul(dst[:], sigo[:], tc_[:])
            else:
                nc.vector.tensor_mul(h[d][:], sigo[:], tc_[:])

        # ---- layer 1: one single cell per dir; A wave = [i0 i1 o0 o1]
        # (one merged sigmoid), B wave = [g0 g1] ----
        def l1_cell(d):
            hA, hB = (h0_f0, h[1]) if d == 0 else (h[0], hb_15)
            g1A = psum.tile([128, 4, NC_W], F32, tag="gp", name=f"g1A{d}")
            g1B = psum.tile([128, 2, NC_W], F32, tag="gp", name=f"g1B{d}")
            # M-tiles in wih1 col order: i0 i1 o0 o1 g0 g1. Two passes:
            # [ones + early-h] runs during the scan tail; the late-h
            # (scan-final) matmuls go last so they don't block the PE queue.
            (kt_e, h_e), (kt_l, h_l) = (((0, hA), (1, hB)) if d == 0
                                        else ((1, hB), (0, hA)))
            for gp_, ms in ((g1A, (0, 1, 2, 3)), (g1B, (4, 5))):
                for pos, m in enumerate(ms):
                    nc.tensor.matmul(gp_[:, pos, :], wih1o_s(d, m), ones8[:],
                                     start=True, stop=False, perf_mode=PM.DoubleRow)
                    nc.tensor.matmul(gp_[:, pos, :], wih1_s(d, kt_e, m), h_e[:],
                                     start=False, stop=False, perf_mode=PM.DoubleRow)
            for gp_, ms in ((g1B, (4, 5)), (g1A, (0, 1, 2, 3))):
                for pos, m in enumerate(ms):
                    nc.tensor.matmul(gp_[:, pos, :], wih1_s(d, kt_l, m), h_l[:],
                                     start=False, stop=True, perf_mode=PM.DoubleRow)
            tg1 = gpool.tile([128, 2, NC_W], BF16, tag="tg")
            nc.scalar.activation(tg1[:], g1B[:], AF.Tanh)
            # i-half sigmoid first so c1 forms while the o-half runs: the
            # o-sigmoid then covers the ACT wait for tanh(c1)'s input
            sA = gpool.tile([128, 4, NC_W], BF16, tag="sigA")
            nc.scalar.activation(sA[:, 0:2, :], g1A[:, 0:2, :], AF.Sigmoid)
            c1 = gpool.tile([128, 2, NC_W], BF16, tag="t1")
            nc.vector.tensor_mul(c1[:], sA[:, 0:2, :], tg1[:])
            nc.scalar.activation(sA[:, 2:4, :], g1A[:, 2:4, :], AF.Sigmoid)
            tc1 = gpool.tile([128, 2, NC_W], BF16, tag="tc")
            nc.scalar.activation(tc1[:], c1[:], AF.Tanh)
            for k in range(2):
                nc.vector.tensor_mul(merged[:, d * 2 + k:d * 2 + k + 1, :],
                                     sA[:, 2 + k:3 + k, :], tc1[:, k:k + 1, :])

        embed_chunk(0, 2, jorder=(1, 0), split_copy=True)
        # ones memsets needed only by layer 1 / out-proj: run on the idle
        # Pool engine so they never block the DVE prologue chain
        nc.gpsimd.memset(ones8[:, 0, :], 1.0)
        nc.gpsimd.memset(ones8[:, 1, :], 0.0)
        nc.gpsimd.memset(ones_b[:], 1.0)
        embed_chunk(2, 2)
        for t in range(T):
            for d in (1, 0):          # bwd first: its final h frees layer-1 d=0
                pend = scan_dir(t, d)
                flush_tail()          # previous dir's tail
                pending.append((t, d, pend))
                if t == T - 1 and d == 1:
                    flush_tail()      # bwd final tail early: frees layer-1 d=0
                if d == 1 and t in (0, 2, 4):
                    embed_chunk(4 + 4 * (t // 2), 4)
            if t == 0:
                for _ in range(len(pending)):
                    flush_tail()
        # pending = [(15, 0)]; layer-1 d=0 needs only h[1] (bwd final, done)
        flush_tail()                  # fwd final tail -> h[0] FIRST: its
                                      # tanh fills the ACT gap while l1-d0
                                      # waits on h[1], and h[0] unblocks
                                      # l1-d1's matmuls ~4us earlier
        l1_cell(0)
        # out projection m-tiles: [ones, k0, k1] (dir-0 merged) can fill
        # while layer-1 d=1 runs; [k2, k3] + copy + DMA after.
        po = psum.tile([128, 2, NC_W], F32, tag="gp")
        l1_cell(1)
        for m in range(2):
            nc.tensor.matmul(po[:, m, :], w_outo[:, m * 128:(m + 1) * 128],
                             ones_b[:], start=True, stop=False)
            for k in range(2):
                nc.tensor.matmul(po[:, m, :], wout_s(k, m), merged[:, k, :],
                                 start=False, stop=False)
        ob = spool.tile([128, 2, NC_W], F32)
        # all matmuls first (no copy-read WAR stall), then the two copies in
        # parallel on ACT/DVE, then the two DMAs on different queues
        for m in range(2):
            for k in range(2, 4):
                nc.tensor.matmul(po[:, m, :], wout_s(k, m), merged[:, k, :],
                                 start=False, stop=(k == 3))
        nc.scalar.copy(ob[:, 0, :], po[:, 0, :])
        nc.vector.tensor_copy(ob[:, 1, :], po[:, 1, :])
        nc.scalar.dma_start(out_d[:, 0, :], ob[:, 0, :])
        nc.sync.dma_start(out_d[:, 1, :], ob[:, 1, :])

    _legalize_waits(nc)
    return nc


_NC_CACHE = None


def kernel(**inputs):
    global _NC_CACHE
    if _NC_CACHE is None:
        _NC_CACHE = build_nc()
    nc = _NC_CACHE

    packed = _pack_host(inputs)
    char_ids = np.asarray(inputs["char_ids"])
    in_maps = []
    for cc in range(NCORES):
        ids_c = char_ids.reshape(B * S, T)[cc * NC_W:(cc + 1) * NC_W]  # [512, 16]
        ids_em = np.ascontiguousarray(ids_c[:, EO].T).astype(np.float16).reshape(TOK)
        in_maps.append({**packed, "ids": ids_em})

    res = run_bass_kernel_spmd(nc, in_maps, list(range(NCORES)))

    outs = []
    for cc in range(NCORES):
        o = res.results[cc]["out"]                 # [128, 2, 512]: feat = m*128+p
        outs.append(o.transpose(1, 0, 2).reshape(256, NC_W).T)   # [512, 256]
    full = np.concatenate(outs, 0)                 # [4096, 256]
    return full.reshape(B, S, H).astype(np.float32)
